# revision 59
# baseline (speedup 1.0000x reference)
"""Trainium2 Bass kernel: atrous (dilated) multi-head attention block.

Computation (per reference):
  x [2, 4096, 1024] --atrous regroup (dil=4)--> xr [8, 1024, 1024]
  q/k/v = xr @ W{q,k,v}.T + b;  16 heads, dh=64
  probs = softmax(q k^T / 8);  ctx = probs v
  atted = ctx @ Wf.T + bf;  final = LN(atted + x)
  returns (final, atted)

Sharding: B*dil == 8 == n_cores, so each NeuronCore takes one atrous group
[1024, 1024] — pure data parallel, zero collectives. The host performs the
strided regroup/scatter and pre-transposes / pre-scales operands so the
device kernel needs no on-chip transposes.

Per-core schedule (all matmuls contract over the partition dim):
  xT/wq/wk [P, MT, KC, P] bf16 m-block-major: the first V-proj group gates
      on only xT's m=0 block + the t=0 half of wv (~1.25MB instead of 3MB),
      and each dma_start costs ~0.6us of Sync issue time, so startup uses
      few large transfers (one DMA's packets already spread across the 16
      SDMA engines).
  qT/kTe/kTo [P, L] bf16 per chunk: head-transposed; scoresT_h = kT_h^T @
      qT_h.  kT is stored per head zero-padded to 128 contraction rows so
      scores matmuls run K=128 like every other matmul — mixing 64-row and
      128-row PE tiling modes drains the systolic array on every switch
      (and every PE gap also drops the clock from 2.4 to 1.2GHz for ~3us,
      so the whole schedule is built to keep the PE stream gap-free).
  expT = exp(scoresT) on ScalarE (Wq pre-scaled by 1/8 on host).  ScalarE
      is the in-stage pacer: exp of one [128,1024] tile (~1.1us) outruns
      the 0.85us of score matmuls per key chunk, so every stage interleaves
      INDEPENDENT filler matmuls (the next chunk's q/k projections, the
      V-proj t=1 half, or the first out-proj groups) into the emission.
  v_aug [L, H, 65] bf16 : v with an appended ones-column per head, so the
      ctx matmul (lhsT=v_aug, rhs=expT) also yields softmax denominators in
      psum row 64.  Normalization: psum-freeing copy on DVE, then the
      denominator row is reshaped to [128, 4] via a DRAM bounce for an
      all-partition DVE reciprocal and broadcast back to the 64 ctx
      partitions.
  Each attention stage runs ctx in TWO passes: pass A streams scores+exp
      with only the t=0 ctx groups (2 psum banks), leaving cpool slots for
      4-per-iteration fillers; pass B runs the t=1 groups at stage end as
      pure catch-up PE work once their whole eT input is exp'd.  Stage 7's
      pass-B filler is the out-projection's m=0 kc<=6 accumulation, so the
      out phase starts before the last norm chains close.
  atted = ctxT^T @ WfT (K=D, wf t-half-major); +bias on DVE; LN epilogue
      via bn_stats/bn_aggr; atted/final staged in one full-size SBUF tile
      (wk's dead wpool buf) and written as 512KB batched DMAs from the
      Activation queue — big DMAs sustain ~2x the BW of per-chunk ones,
      and partition-split transfers just serialize on the same SDMA rings.
"""

import os
import sys
from contextlib import ExitStack

for _p in ("/opt/trn_rl_repo",):
    if os.path.isdir(_p) and _p not in sys.path:
        sys.path.insert(0, _p)

import numpy as np
import ml_dtypes

import concourse.bass as bass
import concourse.mybir as mybir
from concourse.tile import TileContext
from concourse.bass_utils import run_bass_kernel_spmd

B, S, D = 2, 4096, 1024
DIL = 4
NCORES = 8
L = S // DIL  # 1024 rows per core
H, DH = 16, 64
P = 128
KC = D // P  # 8 contraction chunks
MT = D // P  # 8 output chunks
NT = 512  # matmul free-dim tile
NBLK = 16  # partition-block DMA splits (one per DMA engine)
PB = P // NBLK
EPS = 1e-5
SCALE = 1.0 / 8.0  # 1/sqrt(dh)

F32 = mybir.dt.float32
BF16 = mybir.dt.bfloat16
AL = mybir.AluOpType
AF = mybir.ActivationFunctionType
BF16_NP = ml_dtypes.bfloat16


def _split_excess_waits(nc: bass.Bass, max_waits: int = 1) -> None:
    """This neuronxcc's walrus rejects instructions carrying more than
    `max_waits` semaphore waits ("Too many sync wait commands").  Tile's
    kernel-tail drain (and occasionally a compute op) can exceed that.
    Move the excess waits onto same-engine no-ops inserted just before the
    instruction — the engine executes in order, so the happens-before
    relation is preserved exactly."""
    n = 0
    for fn in nc.m.functions:
        for blk in fn.blocks:
            insts = list(blk.instructions)
            out = []
            changed = False
            for inst in insts:
                si = inst.sync_info
                waits = list(si.on_wait) if (si is not None and si.on_wait) else []
                if len(waits) > max_waits:
                    changed = True
                    excess, keep = waits[:-max_waits], waits[-max_waits:]
                    for i in range(0, len(excess), max_waits):
                        nop = mybir.InstNoOp(name=f"waitsplit-{n}", ins=[], outs=[])
                        n += 1
                        nop.engine = inst.engine
                        nop.sync_info = mybir.SyncInfo(
                            on_wait=excess[i : i + max_waits], on_update=[]
                        )
                        nc.register_instruction(nop)
                        out.append(nop)
                    si.on_wait = keep
                out.append(inst)
            if changed:
                blk.instructions = out


def build_graph(apply_affine: bool = False) -> bass.Bass:
    nc = bass.Bass()
    # xT / wq / wk are m-block-major [P, MT, KC, P]: the first V-proj group
    # needs only xT's m=0 block + the t=0 half of wv, shrinking the startup
    # DMA gate from 3MB to ~1.25MB
    xT_e = nc.declare_dram_parameter("xT", [P, MT, KC, P], BF16, isOutput=False)
    xn_e = nc.declare_dram_parameter("xn", [P, MT, D], BF16, isOutput=False)
    wq_e = nc.declare_dram_parameter("wqT", [P, MT, KC, P], BF16, isOutput=False)
    wk_e = nc.declare_dram_parameter("wkT", [P, MT, KC, P], BF16, isOutput=False)
    # wv is t-half-major so V-proj's t=0 groups can start after only half
    # of wv has landed (shrinks the startup DMA gate by ~1MB)
    wv_e = nc.declare_dram_parameter("wvT", [P, 2, KC, NT], BF16, isOutput=False)
    wf_e = nc.declare_dram_parameter("wfT", [P, 2, KC, NT], BF16, isOutput=False)
    bqc_e = nc.declare_dram_parameter("bqc", [P, MT], F32, isOutput=False)
    bkc_e = nc.declare_dram_parameter("bkc", [P, MT], F32, isOutput=False)
    bv_e = nc.declare_dram_parameter("bv", [D], F32, isOutput=False)
    bf_e = nc.declare_dram_parameter("bf", [D], F32, isOutput=False)
    gam_e = nc.declare_dram_parameter("gam", [D], F32, isOutput=False)
    bet_e = nc.declare_dram_parameter("bet", [D], F32, isOutput=False)
    out_e = nc.declare_dram_parameter("out", [2, L, D], BF16, isOutput=True)

    with TileContext(nc) as tc, ExitStack() as ctx:
        const = ctx.enter_context(tc.tile_pool(name="const", bufs=1))
        persist = ctx.enter_context(tc.tile_pool(name="persist", bufs=1))
        wpool = ctx.enter_context(tc.tile_pool(name="wpool", bufs=3))
        epool = ctx.enter_context(tc.tile_pool(name="epool", bufs=2))
        # PSUM: spool 2x[P,L] (4 banks) + cpool 4x[P,NT] (4) = 8 banks.
        # cpool serves ALL [P,NT] accumulation groups (V/QK/ctx/out-proj);
        # the emission order is arranged so each allocation's FIFO buf was
        # freed by a group that closes before the allocator needs it.
        cpool = ctx.enter_context(tc.tile_pool(name="cpool", bufs=4, space="PSUM"))
        spool = ctx.enter_context(tc.tile_pool(name="spool", bufs=2, space="PSUM"))
        npool = ctx.enter_context(tc.tile_pool(name="npool", bufs=2))
        cnpool = ctx.enter_context(tc.tile_pool(name="cnpool", bufs=3))
        dpool = ctx.enter_context(tc.tile_pool(name="dpool", bufs=2, space="DRAM"))
        opool = ctx.enter_context(tc.tile_pool(name="opool", bufs=2))
        apool = ctx.enter_context(tc.tile_pool(name="apool", bufs=2))
        stat = ctx.enter_context(tc.tile_pool(name="stat", bufs=4))

        def dma_blocks(dst, src):
            # one DMA per middle-dim chunk, spanning all 128 partitions:
            # [128, 1, 2-4KB] descriptors sustain ~190+ GB/s across queues
            # (8-partition blocks with long runs measured 3x slower).
            nchunk = dst.shape[1]
            for c in range(nchunk):
                nc.sync.dma_start(out=dst[:, c : c + 1], in_=src[:, c : c + 1])

        # ---- whole-kernel inputs.  v_group(0, 0) gates on the t=0 half of
        # wv plus ONLY xT's m=0 block (m-block-major layout), so the first
        # matmul can start after ~1.25MB instead of 3MB.  Later m blocks
        # stream in behind, one block ahead of the PE's v_group pace.
        xT_sb = persist.tile([P, MT, KC, P], BF16, tag="xT")
        wv_sb = wpool.tile([P, 2, KC, NT], BF16, tag="w", name="wv")
        wq_sb = wpool.tile([P, MT, KC, P], BF16, tag="w", name="wq")
        wk_sb = wpool.tile([P, MT, KC, P], BF16, tag="w", name="wk")

        def dma_mblock(sb, e, m):
            nc.sync.dma_start(out=sb[:, m : m + 1], in_=e[:, m : m + 1])

        # gate: each dma_start costs ~0.6us of Sync issue time, so the
        # startup uses FEW LARGE transfers (packets of one DMA already
        # spread across the SDMA engines): wv-t0 (1MB), xT m0/m1, wq/wk m0
        # — then the rest in big m-interleaved chunks so every consumer
        # stays ~1 block ahead of the PE
        nc.sync.dma_start(out=wv_sb[:, 0:1], in_=wv_e[:, 0:1])
        dma_mblock(xT_sb, xT_e, 0)
        dma_mblock(xT_sb, xT_e, 1)
        dma_mblock(wq_sb, wq_e, 0)
        dma_mblock(wk_sb, wk_e, 0)
        # bvb next: the first v_group drain (~18us) reads it, and a late
        # arrival backs up cpool psum recycling behind the DVE drains
        bvb = const.tile([P, D], F32, tag="bvb")
        for c in range(2):
            nc.sync.dma_start(
                out=bvb[:, c * NT : (c + 1) * NT],
                in_=bv_e[None, c * NT : (c + 1) * NT].to_broadcast((P, NT)),
            )
        nc.sync.dma_start(out=xT_sb[:, 2:4], in_=xT_e[:, 2:4])
        nc.sync.dma_start(out=xT_sb[:, 4:6], in_=xT_e[:, 4:6])
        nc.sync.dma_start(out=xT_sb[:, 6:8], in_=xT_e[:, 6:8])

        bqc_sb = const.tile([P, MT], F32, tag="bqc")
        nc.sync.dma_start(out=bqc_sb[:], in_=bqc_e[:])
        bkc_sb = const.tile([P, MT], F32, tag="bkc")
        nc.sync.dma_start(out=bkc_sb[:], in_=bkc_e[:])
        bfb = const.tile([P, D], F32, tag="bfb")
        if apply_affine:
            gmb = const.tile([P, D], F32, tag="gmb")
            nc.sync.dma_start(out=gmb[:], in_=gam_e[None, :].to_broadcast((P, D)))
            btb = const.tile([P, D], F32, tag="btb")
            nc.sync.dma_start(out=btb[:], in_=bet_e[None, :].to_broadcast((P, D)))
        epsb = const.tile([P, 1], F32, tag="epsb")
        nc.vector.memset(epsb[:], EPS)

        # per-chunk persistent arrays (separate tiles => fine-grained deps)
        qT = [persist.tile([P, L], BF16, tag=f"qT{m}", name=f"qT{m}") for m in range(MT)]
        # kT per head, zero-padded to full 128 contraction rows: scores
        # matmuls then run K=128 like everything else, so the PE never
        # switches tiling mode (each 64-row/128-row mode switch drains the
        # systolic array — interleaved emission measured ~750ns/matmul).
        kTe = [persist.tile([P, L], BF16, tag=f"kTe{m}", name=f"kTe{m}") for m in range(MT)]
        kTo = [persist.tile([P, L], BF16, tag=f"kTo{m}", name=f"kTo{m}") for m in range(MT)]
        vA = [persist.tile([P, H, DH + 1], BF16, tag=f"vA{m}", name=f"vA{m}") for m in range(KC)]
        cT = [persist.tile([P, L], BF16, tag=f"cT{m}", name=f"cT{m}") for m in range(KC)]
        # memsets on the idle GpSimd queue (on DVE they back up behind the
        # early v-drains and stall qk(0)/scores(0)).  vA ones-columns FIRST:
        # stage 0's ctx matmuls read every vA[jc] from ~26us, while
        # kTe/kTo[m] zero-padding is only needed from stage m on.
        for m in range(KC):
            nc.gpsimd.memset(vA[m][:, :, DH : DH + 1], 1.0)
        for m in range(MT):
            nc.gpsimd.memset(kTe[m][DH:P, :], 0.0)
            nc.gpsimd.memset(kTo[m][0:DH, :], 0.0)

        # remaining wq/wk blocks in large m-interleaved chunks
        nc.sync.dma_start(out=wq_sb[:, 1:4], in_=wq_e[:, 1:4])
        nc.sync.dma_start(out=wk_sb[:, 1:4], in_=wk_e[:, 1:4])
        nc.sync.dma_start(out=wq_sb[:, 4:8], in_=wq_e[:, 4:8])
        nc.sync.dma_start(out=wk_sb[:, 4:8], in_=wk_e[:, 4:8])
        # wv's t=1 half (heads 8-15) is not consumed until stage 4 — load it
        # after the qk weights so it never gates the attention start
        nc.sync.dma_start(out=wv_sb[:, 1:2], in_=wv_e[:, 1:2])

        # ---- emission helpers ------------------------------------------
        # v/qk projections are expressed as FILLER GENERATORS: one callable
        # per matmul, fed into the attention stages' emission loops wherever
        # the in-order PE would otherwise wait on the ScalarE exp stream.
        def v_fillers(m, t):
            """V-projection group for (m, t) as 8 one-matmul callables."""
            box = [None]
            for kc in range(KC):
                def mm(kc=kc, box=box, m=m, t=t):
                    if kc == 0:
                        box[0] = cpool.tile([P, NT], F32, tag="cc", name="psv")
                    ps = box[0]
                    nc.tensor.matmul(
                        ps[:],
                        xT_sb[:, m, kc, :],
                        wv_sb[:, t, kc, :],
                        start=(kc == 0),
                        stop=(kc == KC - 1),
                    )
                    if kc == KC - 1:
                        nc.vector.tensor_tensor(
                            vA[m][:, t * 8 : (t + 1) * 8, 0:DH],
                            ps[:].rearrange("p (h e) -> p h e", e=DH),
                            bvb[:, t * NT : (t + 1) * NT].rearrange(
                                "p (h e) -> p h e", e=DH
                            ),
                            AL.add,
                        )
                yield mm

        def qk_fillers(m):
            """Q/K projections for chunk m as 32 one-matmul callables;
            bias-add + bf16 cast on DVE at each group close (keeps ScalarE
            free for exp).  The K psum is split into the two zero-padded
            per-head tiles."""
            for w_sb, bias_sb, dst in ((wq_sb, bqc_sb, None), (wk_sb, bkc_sb, "k")):
                for t in range(2):
                    box = [None]
                    for kc in range(KC):
                        def mm(w_sb=w_sb, bias_sb=bias_sb, dst=dst, t=t, kc=kc, box=box, m=m):
                            if kc == 0:
                                box[0] = cpool.tile([P, NT], F32, tag="cc", name="psqk")
                            ps = box[0]
                            nc.tensor.matmul(
                                ps[:],
                                w_sb[:, m, kc, :],
                                xT_sb[:, 4 * t : 4 * (t + 1), kc, :],
                                start=(kc == 0),
                                stop=(kc == KC - 1),
                            )
                            if kc == KC - 1:
                                sl = slice(t * NT, (t + 1) * NT)
                                if dst is None:
                                    nc.vector.tensor_scalar(
                                        qT[m][:, sl], ps[:], bias_sb[:, m : m + 1], None, AL.add
                                    )
                                else:
                                    nc.vector.tensor_scalar(
                                        kTe[m][0:DH, sl], ps[0:DH, :], bias_sb[0:DH, m : m + 1], None, AL.add
                                    )
                                    nc.vector.tensor_scalar(
                                        kTo[m][DH:P, sl], ps[DH:P, :], bias_sb[DH:P, m : m + 1], None, AL.add
                                    )
                        yield mm

        def drain(gen):
            for f in gen:
                f()

        def scores_jc(hc, jc, eA, eB):
            """Scores+exp for both heads of chunk hc at key-chunk jc, as
            full-K=128 matmuls against the zero-padded per-head kT tiles
            (no PE tiling-mode switches)."""
            psA = spool.tile([P, L], F32, tag="sc", name="scA")
            psB = spool.tile([P, L], F32, tag="sc", name="scB")
            for t in range(2):
                nc.tensor.matmul(
                    psA[:, t * NT : (t + 1) * NT],
                    kTe[hc][:, jc * P : (jc + 1) * P],
                    qT[hc][:, t * NT : (t + 1) * NT],
                    start=True,
                    stop=True,
                )
                nc.tensor.matmul(
                    psB[:, t * NT : (t + 1) * NT],
                    kTo[hc][:, jc * P : (jc + 1) * P],
                    qT[hc][:, t * NT : (t + 1) * NT],
                    start=True,
                    stop=True,
                )
            nc.scalar.activation(eA[:, jc, :], psA[:], AF.Exp)
            nc.scalar.activation(eB[:, jc, :], psB[:], AF.Exp)

        def ctx_mm(h, t, eT, pc, jc):
            nc.tensor.matmul(
                pc[0 : DH + 1, :],
                vA[jc][:, h, :],
                eT[:, jc, t * NT : (t + 1) * NT],
                start=(jc == 0),
                stop=(jc == KC - 1),
            )

        def ctx_norm(h, t, pc):
            """psum -> normalized cT slice.  Denominators sit in psum row 64.
            The psum-freeing copy runs on ScalarE (idle exactly when norms
            close, at stage tails — on DVE it queues behind the qk drains and
            blocks cpool recycling); the denominator row gets a single-op
            approx reciprocal (18 good bits feeding a bf16 product; den >=
            L*exp(-2.5) so no edge cases), broadcast to the 64 ctx partitions
            by one SBUF->SBUF stride-0 DMA."""
            hc, hh = h // 2, h % 2
            hp = hh * DH
            cn = cnpool.tile([DH + 1, NT], F32, tag="cn")
            nc.vector.tensor_copy(out=cn[:], in_=pc[0 : DH + 1, :])
            rdA = dpool.tile([1, NT], F32, tag="rdA")
            nc.sync.dma_start(out=rdA[:], in_=cn[DH : DH + 1, :])
            st = npool.tile([P, NT // P], F32, tag="st")
            nc.sync.dma_start(
                out=st[:], in_=rdA[0, :].rearrange("(p f) -> p f", p=P)
            )
            stR = npool.tile([P, NT // P], F32, tag="stR")
            nc.vector.reciprocal(stR[:], st[:])
            rdB = dpool.tile([1, NT], F32, tag="rdB")
            nc.sync.dma_start(
                out=rdB[0, :].rearrange("(p f) -> p f", p=P), in_=stR[:]
            )
            rb = npool.tile([DH, NT], F32, tag="rb")
            nc.sync.dma_start(out=rb[:], in_=rdB[:].to_broadcast((DH, NT)))
            nc.vector.tensor_tensor(
                cT[hc][hp : hp + DH, t * NT : (t + 1) * NT],
                cn[0:DH, :],
                rb[:],
                AL.mult,
            )

        def emit_stage(hc, filler=None, fillerB=None):
            """One attention stage: scores+exp+ctx for head pair (2hc,
            2hc+1), jc-pipelined in TWO ctx passes.  Pass A streams scores+
            exp with only the t=0 ctx groups (2 psum banks), leaving 2
            cpool slots for `filler` matmuls (the NEXT chunk's q/k
            projections) interleaved 4-per-iteration — independent PE work
            that hides the ScalarE exp pacing (exp of one [128,1024] tile
            is ~1.1us vs 0.85us of score matmuls).  Pass B runs the t=1 ctx
            groups at stage end, when their whole eT input has already been
            exp'd, as pure catch-up PE work."""
            if filler is None:
                filler = iter(())
            eA = epool.tile([P, KC, L], BF16, tag="eT", name=f"eT{2 * hc}")
            eB = epool.tile([P, KC, L], BF16, tag="eT", name=f"eT{2 * hc + 1}")
            pcs = {}

            def fill(n):
                for _ in range(n):
                    f = next(filler, None)
                    if f is None:
                        return
                    f()

            def ctx(h, t, eT, c):
                if 0 <= c < KC:
                    if c == 0:
                        pcs[(h, t)] = cpool.tile(
                            [P, NT], F32, tag="cc", name=f"pc{h}_{t}"
                        )
                    ctx_mm(h, t, eT, pcs[(h, t)], c)
                    if c == KC - 1:
                        ctx_norm(h, t, pcs.pop((h, t)))

            LAG = 2
            for i in range(KC + LAG + 2):
                fill(2)
                if i < KC:
                    scores_jc(hc, i, eA, eB)
                fill(2)
                ctx(2 * hc, 0, eA, i - LAG)
                ctx(2 * hc + 1, 0, eB, i - LAG - 1)
            if fillerB is not None:
                filler = chain(fillerB, filler)
            for i in range(KC + 2):
                ctx(2 * hc, 1, eA, i)
                ctx(2 * hc + 1, 1, eB, i - 1)
                fill(2)
            for f in filler:
                f()

        # ---- emission ---------------------------------------------------
        # V projection t=0 standalone first: ctx chunk jc reads vA[jc], so
        # all of V must precede the first stage's ctx half anyway, and the
        # PE runs it gap-free while the ScalarE pipeline is still empty.
        for m in range(MT):
            drain(v_fillers(m, 0))
        drain(qk_fillers(0))
        # V-proj t=1 (heads 8-15, first read by stage 4's ctx) is filler for
        # stages 0-2 alongside each next chunk's q/k projections.
        from itertools import chain

        emit_stage(0, chain(qk_fillers(1), v_fillers(0, 1), v_fillers(1, 1), v_fillers(2, 1)))
        emit_stage(1, chain(qk_fillers(2), v_fillers(3, 1), v_fillers(4, 1), v_fillers(5, 1)))
        emit_stage(2, chain(qk_fillers(3), v_fillers(6, 1), v_fillers(7, 1)))
        xn_sb = wpool.tile([P, MT, D], BF16, tag="w", name="xn")  # reuses wv slot
        dma_blocks(xn_sb, xn_e)
        for hc in range(3, KC):
            if hc == KC - 2:
                # bfb is first read by out_m(0); load it here so the 512KB
                # broadcast never contends with the startup gate
                for c in range(4):
                    nc.sync.dma_start(
                        out=bfb[:, c * 256 : (c + 1) * 256],
                        in_=bf_e[None, c * 256 : (c + 1) * 256].to_broadcast((P, 256)),
                    )
            if hc == KC - 1:
                # wf t-half-major like wv: the out-proj t=0 groups start
                # after only half of wf has landed
                wf_sb = wpool.tile([P, 2, KC, NT], BF16, tag="w", name="wf")
                for t in range(2):
                    nc.sync.dma_start(out=wf_sb[:, t : t + 1], in_=wf_e[:, t : t + 1])
            if hc < KC - 1:
                emit_stage(hc, qk_fillers(hc + 1))
            else:
                # stage 7's pass-B filler: out-proj m=0 groups over kc 0..6
                # (kc=7 needs this stage's own cT t0-norms, finished after
                # pass A) — the out phase starts ~7us before the stage ends
                out_pre = {}

                def out_open_kc6(m, t):
                    box = [None]
                    out_pre[(m, t)] = box
                    for kc in range(KC - 1):
                        def mm(m=m, t=t, kc=kc, box=box):
                            if kc == 0:
                                box[0] = cpool.tile([P, NT], F32, tag="cc", name="po")
                            nc.tensor.matmul(
                                box[0][:],
                                cT[kc][:, m * P : (m + 1) * P],
                                wf_sb[:, t, kc, :],
                                start=(kc == 0),
                                stop=False,
                            )
                        yield mm

                emit_stage(hc, None, fillerB=chain(out_open_kc6(0, 0), out_open_kc6(0, 1)))

        # ---- output projection + LN.  atted/final for pairs of m chunks
        # are staged in WIDE tiles and written with one 512KB dma_start per
        # pair per tensor (big DMAs sustain far higher BW than per-chunk
        # 256KB ones, and two half-partition transfers serialize on the same
        # SDMA rings anyway).  Output DMAs issue from the Activation queue
        # (HWDGE, same engines) — ScalarE idles here while Sync still runs
        # the ctx_norm bounce chains of the last stage.
        # single full-size staging tile in wk's wpool buf (wk is dead after
        # the qk(7) fillers in stage 6): atted slices are written, DMA'd per
        # batch, then the SAME columns take the LN output for the final
        # DMA — zero extra SBUF, and 512KB DMAs instead of 16 x 256KB.
        # The last two chunks go as singles so the final LN chain of m=6
        # overlaps m=7's matmuls instead of serializing after them.
        attw = wpool.tile([P, MT, D], BF16, tag="w", name="attw")

        def out_batch(ms):
            sl = slice(ms[0] * P, (ms[-1] + 1) * P)
            for m in ms:
                for t in range(2):
                    pre = out_pre.pop((m, t), None)
                    if pre is not None and pre[0] is not None:
                        ps = pre[0]
                        nc.tensor.matmul(
                            ps[:],
                            cT[KC - 1][:, 0:P],
                            wf_sb[:, t, KC - 1, :],
                            start=False,
                            stop=True,
                        )
                    else:
                        ps = cpool.tile([P, NT], F32, tag="cc", name="po")
                        for kc in range(KC):
                            nc.tensor.matmul(
                                ps[:],
                                cT[kc][:, m * P : (m + 1) * P],
                                wf_sb[:, t, kc, :],
                                start=(kc == 0),
                                stop=(kc == KC - 1),
                            )
                    nc.vector.tensor_tensor(
                        attw[:, m, t * NT : (t + 1) * NT],
                        ps[:],
                        bfb[:, t * NT : (t + 1) * NT],
                        AL.add,
                    )
            nc.scalar.dma_start(
                out=out_e[1, sl, :].rearrange("(b p) d -> p b d", p=P),
                in_=attw[:, ms[0] : ms[-1] + 1, :],
            )
            for m in ms:
                # LN epilogue: mean+var in one bn_stats pass (free dim capped
                # at 512, so two sub-batches aggregated by bn_aggr)
                res = opool.tile([P, D], BF16, tag="res")
                nc.vector.tensor_tensor(res[:], attw[:, m, :], xn_sb[:, m, :], AL.add)
                bst = stat.tile([P, 2, 6], F32, tag="bst")
                nc.vector.bn_stats(bst[:, 0, :], res[:, 0:NT])
                nc.vector.bn_stats(bst[:, 1, :], res[:, NT:D])
                mv = stat.tile([P, 2], F32, tag="mv")
                nc.vector.bn_aggr(mv[:], bst[:])
                sd = stat.tile([P, 1], F32, tag="sd")
                nc.scalar.activation(sd[:], mv[:, 1:2], AF.Sqrt, bias=epsb[:])
                inv = stat.tile([P, 1], F32, tag="inv")
                nc.vector.reciprocal(inv[:], sd[:])
                mu = mv[:, 0:1]
                if apply_affine:
                    nc.vector.tensor_scalar(res[:], res[:], mu, inv[:], AL.subtract, AL.mult)
                    nc.vector.scalar_tensor_tensor(res[:], res[:], 1.0, gmb[:], AL.mult, AL.mult)
                    nc.vector.tensor_tensor(attw[:, m, :], res[:], btb[:], AL.add)
                else:
                    nc.vector.tensor_scalar(attw[:, m, :], res[:], mu, inv[:], AL.subtract, AL.mult)
            nc.scalar.dma_start(
                out=out_e[0, sl, :].rearrange("(b p) d -> p b d", p=P),
                in_=attw[:, ms[0] : ms[-1] + 1, :],
            )

        for ms in ((0, 1), (2, 3), (4, 5), (6,), (7,)):
            out_batch(ms)

    _split_excess_waits(nc)
    return nc


def prepare_in_maps(inputs):
    def chunk_pmajor(a):
        # [KC*P, N] -> [P, KC, N] so each partition's row is contiguous HBM
        kcp, n = a.shape
        return np.ascontiguousarray(
            a.reshape(KC, P, n).transpose(1, 0, 2)
        )

    def mblock_major(a):
        # [KC*P, MT*P] -> [P, MT, KC, P]: block (m) of output columns is
        # contiguous per partition so one DMA delivers a full m block
        return np.ascontiguousarray(
            a.reshape(KC, P, MT, P).transpose(1, 2, 0, 3)
        )

    x = np.asarray(inputs["x"], np.float32)
    xr = x.reshape(B, L, DIL, D).transpose(0, 2, 1, 3).reshape(NCORES, L, D)
    shared = {
        "wqT": mblock_major((np.asarray(inputs["Wq"], np.float32).T * SCALE).astype(BF16_NP)),
        "wkT": mblock_major(np.asarray(inputs["Wk"], np.float32).T.astype(BF16_NP)),
        "wvT": np.ascontiguousarray(
            chunk_pmajor(np.asarray(inputs["Wv"], np.float32).T.astype(BF16_NP))
            .reshape(P, KC, 2, NT)
            .transpose(0, 2, 1, 3)
        ),
        "wfT": np.ascontiguousarray(
            chunk_pmajor(np.asarray(inputs["Wf"], np.float32).T.astype(BF16_NP))
            .reshape(P, KC, 2, NT)
            .transpose(0, 2, 1, 3)
        ),
        "bqc": np.ascontiguousarray(
            (np.asarray(inputs["bq"], np.float32) * SCALE).reshape(MT, P).T
        ),
        "bkc": np.ascontiguousarray(
            np.asarray(inputs["bk"], np.float32).reshape(MT, P).T
        ),
        "bv": np.ascontiguousarray(inputs["bv"], dtype=np.float32),
        "bf": np.ascontiguousarray(inputs["bf"], dtype=np.float32),
        "gam": np.ascontiguousarray(inputs["gamma"], dtype=np.float32),
        "bet": np.ascontiguousarray(inputs["beta"], dtype=np.float32),
    }
    maps = []
    for c in range(NCORES):
        xs = np.ascontiguousarray(xr[c])
        m = dict(shared)
        m["xT"] = mblock_major(np.ascontiguousarray(xs.T).astype(BF16_NP))
        m["xn"] = chunk_pmajor(xs.astype(BF16_NP))
        maps.append(m)
    return maps


def gather_outputs(results):
    outs = np.stack(
        [np.asarray(results[c]["out"]).astype(np.float32) for c in range(NCORES)]
    )
    final = outs[:, 0].reshape(B, DIL, L, D).transpose(0, 2, 1, 3).reshape(B, S, D)
    atted = outs[:, 1].reshape(B, DIL, L, D).transpose(0, 2, 1, 3).reshape(B, S, D)
    return np.ascontiguousarray(final), np.ascontiguousarray(atted)


_GRAPHS = {}


def get_graph(apply_affine=False):
    if apply_affine not in _GRAPHS:
        _GRAPHS[apply_affine] = build_graph(apply_affine)
    return _GRAPHS[apply_affine]


def run(inputs, trace=False, **kw):
    # gamma/beta are fixed to ones/zeros by the reference's setup_inputs;
    # only emit the affine LN ops if they are actually non-identity.
    apply_affine = not (
        np.all(np.asarray(inputs["gamma"]) == 1.0)
        and np.all(np.asarray(inputs["beta"]) == 0.0)
    )
    nc = get_graph(apply_affine)
    maps = prepare_in_maps(inputs)
    res = run_bass_kernel_spmd(nc, maps, core_ids=list(range(NCORES)), trace=trace, **kw)
    return gather_outputs(res.results), res


def kernel(**inputs):
    (final, atted), _ = run(inputs, trace=False)
    return final, atted

